# revision 1
# baseline (speedup 1.0000x reference)
"""GCN (2-layer, PyG GCNConv semantics) on 8 NeuronCores.

Strategy (per sharding hint): 1D node partition across 8 cores.
Host-side prep: per-destination edge bucketing (sort edges by dst, pad
to max in-degree D) which turns the segment-sum scatter into a dense
gather + weighted reduction on device. Weight matrices are replicated;
layer-1 activations are exchanged with an all_gather so layer 2 can
gather arbitrary source nodes (halo exchange).
"""

import numpy as np
import jax
import jax.numpy as jnp
from functools import partial

N = 100000
E = 1600000
NFEAT, NHID, NCLASS = 128, 64, 10
NCORES = 8
NP_ = N // NCORES  # 12500 nodes per core


def _bucket_edges(src, dst, norm):
    """Sort edges by destination, pad each node's in-edge list to D.

    Returns src_pad [N, D] int32, norm_pad [N, D] f32 (0 for padding).
    """
    order = np.argsort(dst, kind="stable")
    dsts = dst[order]
    srcs = src[order].astype(np.int32)
    norms = norm[order].astype(np.float32)
    counts = np.bincount(dsts, minlength=N)
    D = int(counts.max())
    D = ((D + 7) // 8) * 8  # round up: fewer distinct compiled shapes
    offsets = np.zeros(N, dtype=np.int64)
    np.cumsum(counts[:-1], out=offsets[1:])
    pos = np.arange(E, dtype=np.int64) - offsets[dsts]
    src_pad = np.zeros((N, D), dtype=np.int32)
    norm_pad = np.zeros((N, D), dtype=np.float32)
    src_pad[dsts, pos] = srcs
    norm_pad[dsts, pos] = norms
    return src_pad, norm_pad


@partial(jax.pmap, axis_name="x",
         in_axes=(None, 0, 0, 0, 0, None, None, None, None),
         out_axes=0)
def _gcn2(x_full, src_pad, norm_pad, selfco, rows, W1, b1, W2, b2):
    # ---- layer 1 (features replicated; each core reduces its own rows)
    h1 = x_full @ W1                               # [N, 64] replicated
    g = jnp.take(h1, src_pad.reshape(-1), axis=0)  # [Np*D, 64]
    g = g.reshape(src_pad.shape[0], src_pad.shape[1], -1)
    agg = jnp.sum(norm_pad[:, :, None] * g, axis=1)        # [Np, 64]
    h1_own = jnp.take(h1, rows, axis=0)                    # [Np, 64]
    x1 = jax.nn.relu(agg + selfco[:, None] * h1_own + b1)  # [Np, 64]
    # ---- halo exchange: every core needs every node's layer-1 output
    x1_full = jax.lax.all_gather(x1, "x").reshape(-1, x1.shape[1])
    # ---- layer 2
    h2 = x1_full @ W2                              # [N, 10] replicated
    g2 = jnp.take(h2, src_pad.reshape(-1), axis=0)
    g2 = g2.reshape(src_pad.shape[0], src_pad.shape[1], -1)
    agg2 = jnp.sum(norm_pad[:, :, None] * g2, axis=1)      # [Np, 10]
    h2_own = jnp.take(h2, rows, axis=0)
    o = agg2 + selfco[:, None] * h2_own + b2
    return jax.nn.log_softmax(o, axis=1)


def kernel(features, edge_index, edge_weight, W1, b1, W2, b2):
    features = np.asarray(features, dtype=np.float32)
    edge_index = np.asarray(edge_index)
    edge_weight = np.asarray(edge_weight, dtype=np.float32)
    src = np.asarray(edge_index[0]).astype(np.int64)
    dst = np.asarray(edge_index[1]).astype(np.int64)

    # Per-edge normalization (PyG GCNConv, symmetric, self-loop wt 1).
    deg = np.bincount(dst, weights=edge_weight, minlength=N).astype(
        np.float32) + 1.0
    dinv = 1.0 / np.sqrt(deg)
    norm = dinv[src] * edge_weight * dinv[dst]
    selfco = (dinv * dinv).astype(np.float32)

    src_pad, norm_pad = _bucket_edges(src, dst, norm)

    # Shard node-partitioned tensors across the 8 cores.
    src_pad_s = src_pad.reshape(NCORES, NP_, -1)
    norm_pad_s = norm_pad.reshape(NCORES, NP_, -1)
    selfco_s = selfco.reshape(NCORES, NP_)
    rows_s = np.arange(N, dtype=np.int32).reshape(NCORES, NP_)

    out = _gcn2(jnp.asarray(features), jnp.asarray(src_pad_s),
                jnp.asarray(norm_pad_s), jnp.asarray(selfco_s),
                jnp.asarray(rows_s), jnp.asarray(np.asarray(W1, np.float32)),
                jnp.asarray(np.asarray(b1, np.float32)),
                jnp.asarray(np.asarray(W2, np.float32)),
                jnp.asarray(np.asarray(b2, np.float32)))
    return np.asarray(out).reshape(N, NCLASS)


# revision 3
# speedup vs baseline: 24.9888x; 24.9888x over previous
"""GCN (2-layer, PyG GCNConv semantics) on 8 NeuronCores.

Strategy (per sharding hint): 1D node partition across 8 cores.
Host-side prep: per-destination edge bucketing (sort edges by dst, pad
to max in-degree D) which turns the segment-sum scatter into a dense
gather + weighted reduction on device. Weight matrices are replicated;
layer-1 activations are exchanged with an all_gather so layer 2 can
gather arbitrary source nodes (halo exchange).
"""

import numpy as np
import jax
import jax.numpy as jnp
from functools import partial

N = 100000
E = 1600000
NFEAT, NHID, NCLASS = 128, 64, 10
NCORES = 8
NP_ = N // NCORES  # 12500 nodes per core


def _bucket_edges(src, dst, norm):
    """Sort edges by destination, pad each node's in-edge list to D.

    Returns src_pad [N, D] int32, norm_pad [N, D] f32 (0 for padding).
    """
    order = np.argsort(dst, kind="stable")
    dsts = dst[order]
    srcs = src[order].astype(np.int32)
    norms = norm[order].astype(np.float32)
    counts = np.bincount(dsts, minlength=N)
    D = int(counts.max())
    D = ((D + 7) // 8) * 8  # round up: fewer distinct compiled shapes
    offsets = np.zeros(N, dtype=np.int64)
    np.cumsum(counts[:-1], out=offsets[1:])
    pos = np.arange(E, dtype=np.int64) - offsets[dsts]
    src_pad = np.zeros((N, D), dtype=np.int32)
    norm_pad = np.zeros((N, D), dtype=np.float32)
    src_pad[dsts, pos] = srcs
    norm_pad[dsts, pos] = norms
    return src_pad, norm_pad


@partial(jax.pmap, axis_name="x",
         in_axes=(0, 0, 0, 0, None, None, None, None),
         out_axes=0)
def _gcn2(x_c, src_pad, norm_pad, selfco, W1, b1, W2, b2):
    # ---- layer 1: local matmul on owned rows, then halo all_gather so
    # every core can gather arbitrary source-node activations.
    h1_c = x_c @ W1                                     # [Np, 64]
    h1 = jax.lax.all_gather(h1_c, "x").reshape(-1, h1_c.shape[1])
    g = jnp.take(h1, src_pad.reshape(-1), axis=0)       # [Np*D, 64]
    g = g.reshape(src_pad.shape[0], src_pad.shape[1], -1)
    agg = jnp.sum(norm_pad[:, :, None] * g, axis=1)     # [Np, 64]
    x1 = jax.nn.relu(agg + selfco[:, None] * h1_c + b1)
    # ---- layer 2: same pattern with the 10-wide activations
    h2_c = x1 @ W2                                      # [Np, 10]
    h2 = jax.lax.all_gather(h2_c, "x").reshape(-1, h2_c.shape[1])
    g2 = jnp.take(h2, src_pad.reshape(-1), axis=0)
    g2 = g2.reshape(src_pad.shape[0], src_pad.shape[1], -1)
    agg2 = jnp.sum(norm_pad[:, :, None] * g2, axis=1)   # [Np, 10]
    o = agg2 + selfco[:, None] * h2_c + b2
    return jax.nn.log_softmax(o, axis=1)


def kernel(features, edge_index, edge_weight, W1, b1, W2, b2):
    features = np.asarray(features, dtype=np.float32)
    edge_index = np.asarray(edge_index)
    edge_weight = np.asarray(edge_weight, dtype=np.float32)
    src = np.asarray(edge_index[0]).astype(np.int64)
    dst = np.asarray(edge_index[1]).astype(np.int64)

    # Per-edge normalization (PyG GCNConv, symmetric, self-loop wt 1).
    deg = np.bincount(dst, weights=edge_weight, minlength=N).astype(
        np.float32) + 1.0
    dinv = 1.0 / np.sqrt(deg)
    norm = dinv[src] * edge_weight * dinv[dst]
    selfco = (dinv * dinv).astype(np.float32)

    src_pad, norm_pad = _bucket_edges(src, dst, norm)

    # Shard node-partitioned tensors across the 8 cores.
    feats_s = features.reshape(NCORES, NP_, NFEAT)
    src_pad_s = src_pad.reshape(NCORES, NP_, -1)
    norm_pad_s = norm_pad.reshape(NCORES, NP_, -1)
    selfco_s = selfco.reshape(NCORES, NP_)

    out = _gcn2(feats_s, src_pad_s, norm_pad_s, selfco_s,
                np.asarray(W1, np.float32), np.asarray(b1, np.float32),
                np.asarray(W2, np.float32), np.asarray(b2, np.float32))
    return np.asarray(out).reshape(N, NCLASS)


# revision 5
# speedup vs baseline: 25.9261x; 1.0375x over previous
"""GCN (2-layer, PyG GCNConv semantics) on 8 NeuronCores.

Strategy (per sharding hint): 1D node partition across 8 cores.
Host-side prep: per-destination edge bucketing (sort edges by dst, pad
to max in-degree D) which turns the segment-sum scatter into a dense
gather + weighted reduction on device. Weight matrices are replicated;
layer-1 activations are exchanged with an all_gather so layer 2 can
gather arbitrary source nodes (halo exchange).
"""

import numpy as np
import jax
import jax.numpy as jnp
from functools import partial

N = 100000
E = 1600000
NFEAT, NHID, NCLASS = 128, 64, 10
NCORES = 8
NP_ = N // NCORES  # 12500 nodes per core


def _bucket_edges(src, dst, norm):
    """Sort edges by destination, pad each node's in-edge list to D.

    Returns src_pad [N, D] int32, norm_pad [N, D] f32 (0 for padding).
    """
    order = np.argsort(dst, kind="stable")
    dsts = dst[order]
    srcs = src[order].astype(np.int32)
    norms = norm[order].astype(np.float32)
    counts = np.bincount(dsts, minlength=N)
    D = int(counts.max())
    D = ((D + 7) // 8) * 8  # round up: fewer distinct compiled shapes
    offsets = np.zeros(N, dtype=np.int64)
    np.cumsum(counts[:-1], out=offsets[1:])
    pos = np.arange(E, dtype=np.int64) - offsets[dsts]
    src_pad = np.zeros((N, D), dtype=np.int32)
    norm_pad = np.zeros((N, D), dtype=np.float32)
    src_pad[dsts, pos] = srcs
    norm_pad[dsts, pos] = norms
    return src_pad, norm_pad


@partial(jax.pmap, axis_name="x",
         in_axes=(0, 0, 0, 0, None, None, None, None),
         out_axes=0)
def _gcn2(x_c, src_pad, norm_pad, selfco, W1, b1, W2, b2):
    # ---- layer 1: local matmul on owned rows, then halo all_gather so
    # every core can gather arbitrary source-node activations.
    h1_c = x_c @ W1                                     # [Np, 64]
    h1 = jax.lax.all_gather(h1_c, "x").reshape(-1, h1_c.shape[1])
    # indices are guaranteed in [0, N): skip clip-mode bounds handling
    g = h1.at[src_pad.reshape(-1)].get(mode="promise_in_bounds")
    g = g.reshape(src_pad.shape[0], src_pad.shape[1], -1)
    agg = jnp.sum(norm_pad[:, :, None] * g, axis=1)     # [Np, 64]
    x1 = jax.nn.relu(agg + selfco[:, None] * h1_c + b1)
    # ---- layer 2: same pattern with the 10-wide activations
    h2_c = x1 @ W2                                      # [Np, 10]
    h2 = jax.lax.all_gather(h2_c, "x").reshape(-1, h2_c.shape[1])
    g2 = h2.at[src_pad.reshape(-1)].get(mode="promise_in_bounds")
    g2 = g2.reshape(src_pad.shape[0], src_pad.shape[1], -1)
    agg2 = jnp.sum(norm_pad[:, :, None] * g2, axis=1)   # [Np, 10]
    o = agg2 + selfco[:, None] * h2_c + b2
    return jax.nn.log_softmax(o, axis=1)


def kernel(features, edge_index, edge_weight, W1, b1, W2, b2):
    features = np.asarray(features, dtype=np.float32)
    edge_index = np.asarray(edge_index)
    edge_weight = np.asarray(edge_weight, dtype=np.float32)
    src = np.asarray(edge_index[0]).astype(np.int64)
    dst = np.asarray(edge_index[1]).astype(np.int64)

    # Per-edge normalization (PyG GCNConv, symmetric, self-loop wt 1).
    deg = np.bincount(dst, weights=edge_weight, minlength=N).astype(
        np.float32) + 1.0
    dinv = 1.0 / np.sqrt(deg)
    norm = dinv[src] * edge_weight * dinv[dst]
    selfco = (dinv * dinv).astype(np.float32)

    src_pad, norm_pad = _bucket_edges(src, dst, norm)

    # Shard node-partitioned tensors across the 8 cores.
    feats_s = features.reshape(NCORES, NP_, NFEAT)
    src_pad_s = src_pad.reshape(NCORES, NP_, -1)
    norm_pad_s = norm_pad.reshape(NCORES, NP_, -1)
    selfco_s = selfco.reshape(NCORES, NP_)

    out = _gcn2(feats_s, src_pad_s, norm_pad_s, selfco_s,
                np.asarray(W1, np.float32), np.asarray(b1, np.float32),
                np.asarray(W2, np.float32), np.asarray(b2, np.float32))
    return np.asarray(out).reshape(N, NCLASS)
